# revision 10
# baseline (speedup 1.0000x reference)
"""Trainium2 Bass kernel for CustomBCELoss.

Reference semantics (per torch BCELoss with per-channel weighting):
    p, t flattened channel-first to (C=3, M=8388608)
    ones[c]   = count_nonzero(t[c])
    weight[c] = M / max(ones[c], 1)  if ones[c] > 0 else 1000.0
    bce[c]    = -mean(t*max(log p, -100) + (1-t)*max(log1p(-p), -100))
    out       = mean(weight * bce)

Since t ∈ {0,1}, the per-element term is log|p + t - 1|, and with
p ∈ [1e-4, 1-1e-4] the -100 clamp never fires: |p + t - 1| >= ~6e-5.

Single-stream encoding: p > 0 always, so its fp32 sign bit is free. The
host packs t there (p'' = +p if t==1 else -p, a lossless re-encoding of
the (p, t) pair), halving the HBM stream to 12.6 MB/core vs streaming
(p, t) separately.

The per-element work is 3 logical passes (u-prep, count, log) over two
elementwise engines (DVE ~114 G elem/s, ACT ~131 G elem/s at fp32; all
2x DVE perf modes are avoided — engaging one measured a 1.2x static
clock derate on ALL engines). The schedule makes every tile
structurally identical and balances the engines at ~3.4 us per
2048-col tile (vs DMA ~2.6-2.9 us, so compute-bound by ~15%):
  DVE: u = |p + t - 1| = (p'' < 0) + p''   -- ONE fused STT with
           src0 == src1 == p_t (the is_lt intermediate is the 1-t
           step); inner rounding 2^-24 -> ~3e-7 relative on the loss.
       u2 = u[:, 0::2] * u[:, 1::2]        -- pair product, f/2 outs.
  ACT: count: Sign(p'') + accum_out. sign ∈ {-1,+1}; the host recovers
           ones = (accum + n)/2 exactly. Sign is in the natural_log
           table set (no table switch) and depends only on the DMA, so
           ACT starts the moment a tile lands.
       Ln(u2) + accum_out over f/2 elements: sum of ln(u*u') equals
           sum of ln(u) exactly in infinite precision (pair-product
           rounding adds ~2^-24 relative per pair, washing out over
           8.4M elements).
PE/PSUM/GpSimd are never touched (fp32r matmuls and GpSimd DMA derate
all engine clocks 1.2x). Tiles open at 1024 cols, cruise at 2048,
taper 1024/1024/1024/512/512. A dummy Ln in the preamble pins the
table set. Results ship in readiness order.
Tiles never cross an (n, c) half-block boundary, so per-tile partials
map 1:1 to channels on the host, which applies the tiny weight/mean
epilogue in float64.
"""

import numpy as np

import concourse.bacc as bacc
import concourse.bass as bass
import concourse.tile as tile
from concourse import mybir
from concourse.bass_utils import run_bass_kernel_spmd

N_CORES = 8
C = 3
SPATIAL = 128 * 128 * 128            # elements per (n, c) block
N_BATCH = 4
FULL = N_BATCH * C * SPATIAL         # 25_165_824 total elements
PER_CORE = FULL // N_CORES           # 3_145_728
P = 128
# Per-partition column counts per tile; sum must equal PER_CORE / P = 24576.
TILE_F = [1024, 2048, 2048, 2048, 1024,
          2048, 2048, 2048, 2048,
          2048, 2048, 1024, 1024, 1024, 512, 512]
NTILES = len(TILE_F)
TILE_ELEMS = [P * f for f in TILE_F]
assert sum(TILE_ELEMS) == PER_CORE
HALF_BLOCK_COLS = (SPATIAL // 2) // P          # 8192 cols per half-block
M_PER_CH = FULL // C                 # 8_388_608
EMPTY_WEIGHT = 1000.0
VS_SPLIT = 13                        # bulk/tail split for the output DMAs

_NC_CACHE = None


def _build_nc():
    nc = bacc.Bacc(
        "TRN2", target_bir_lowering=False, debug=False, num_devices=N_CORES
    )
    p_in = nc.declare_dram_parameter(
        "p_in", [PER_CORE], mybir.dt.float32, isOutput=False
    )
    vsum_out = nc.declare_dram_parameter(
        "vsum", [P, NTILES], mybir.dt.float32, isOutput=True
    )
    cnt_out = nc.declare_dram_parameter(
        "cnt", [P, NTILES], mybir.dt.float32, isOutput=True
    )

    off = 0
    for f in TILE_F:
        assert off // HALF_BLOCK_COLS == (off + f - 1) // HALF_BLOCK_COLS
        off += f

    with tile.TileContext(nc) as tc:
        with (
            tc.tile_pool(name="pp", bufs=10) as p_pool,
            tc.tile_pool(name="up", bufs=5) as u_pool,
            tc.tile_pool(name="u2p", bufs=3) as u2_pool,
            tc.tile_pool(name="dp", bufs=2) as dump_pool,
            tc.tile_pool(name="res", bufs=1) as res_pool,
        ):
            vsum_t = res_pool.tile([P, NTILES], mybir.dt.float32)
            cnt_t = res_pool.tile([P, NTILES], mybir.dt.float32)
            # Dummy Ln pins the natural_log table set in the preamble
            # (it also contains Sign).
            warm_t = res_pool.tile([P, 1], mybir.dt.float32)
            nc.vector.memset(warm_t, 1.0)
            nc.scalar.activation(
                out=warm_t, in_=warm_t, func=mybir.ActivationFunctionType.Ln
            )
            off = 0
            for i, f in enumerate(TILE_F):
                n = P * f
                p_src = p_in[off : off + n].rearrange("(p f) -> p f", p=P)
                off += n
                p_t = p_pool.tile([P, f], mybir.dt.float32, tag="p")
                nc.sync.dma_start(out=p_t, in_=p_src)
                # ACT: accum = sum of sign(p'') = 2*ones_tile - n_tile.
                dump = dump_pool.tile([P, f], mybir.dt.bfloat16, tag="d")
                nc.scalar.activation(
                    out=dump,
                    in_=p_t,
                    func=mybir.ActivationFunctionType.Sign,
                    accum_out=cnt_t[:, i : i + 1],
                )
                # DVE: u = (p'' < 0) + p'', fused STT (src0 == src1).
                u_t = u_pool.tile([P, f], mybir.dt.float32, tag="u")
                nc.vector.scalar_tensor_tensor(
                    out=u_t,
                    in0=p_t,
                    scalar=0.0,
                    in1=p_t,
                    op0=mybir.AluOpType.is_lt,
                    op1=mybir.AluOpType.add,
                )
                # DVE: pair product u2[k] = u[2k] * u[2k+1] (f/2 outputs).
                u_pairs = u_t[:].rearrange("p (f two) -> p f two", two=2)
                u2_t = u2_pool.tile([P, f // 2], mybir.dt.float32, tag="u2")
                nc.vector.tensor_tensor(
                    out=u2_t,
                    in0=u_pairs[:, :, 0],
                    in1=u_pairs[:, :, 1],
                    op=mybir.AluOpType.mult,
                )
                # ACT: accum = sum of ln(u2) = sum of ln(u) for the tile.
                nc.scalar.activation(
                    out=u2_t,
                    in_=u2_t,
                    func=mybir.ActivationFunctionType.Ln,
                    accum_out=vsum_t[:, i : i + 1],
                )
            # Ship results in readiness order so only a tiny vsum chunk
            # trails the last Ln.
            nc.sync.dma_start(
                out=cnt_out[:, :VS_SPLIT], in_=cnt_t[:, :VS_SPLIT]
            )
            nc.sync.dma_start(
                out=vsum_out[:, :VS_SPLIT], in_=vsum_t[:, :VS_SPLIT]
            )
            nc.sync.dma_start(
                out=cnt_out[:, VS_SPLIT:], in_=cnt_t[:, VS_SPLIT:]
            )
            nc.sync.dma_start(
                out=vsum_out[:, VS_SPLIT:], in_=vsum_t[:, VS_SPLIT:]
            )
    nc.compile()
    return nc


def _get_nc():
    global _NC_CACHE
    if _NC_CACHE is None:
        _NC_CACHE = _build_nc()
    return _NC_CACHE


def _pack(input, target):
    """Lossless (p, t) -> p'' re-encoding: t into p's free sign bit."""
    p_flat = np.ascontiguousarray(input, dtype=np.float32).reshape(-1)
    t_flat = np.ascontiguousarray(target, dtype=np.float32).reshape(-1)
    p_bits = p_flat.view(np.uint32)
    sign = np.where(t_flat == 0.0, np.uint32(0x80000000), np.uint32(0))
    return (p_bits | sign).view(np.float32)


def _run_device(input, target, **spmd_kwargs):
    packed = _pack(input, target)
    in_maps = []
    for k in range(N_CORES):
        sl = slice(k * PER_CORE, (k + 1) * PER_CORE)
        in_maps.append({"p_in": packed[sl]})
    return run_bass_kernel_spmd(nc=_get_nc(), in_maps=in_maps,
                                core_ids=list(range(N_CORES)), **spmd_kwargs)


def _epilogue(results):
    sum_v = np.zeros(C, dtype=np.float64)
    ones = np.zeros(C, dtype=np.float64)
    for k in range(N_CORES):
        vs = results[k]["vsum"].astype(np.float64)   # [P, NTILES]
        ct = results[k]["cnt"].astype(np.float64)    # [P, NTILES]
        off = 0
        for i, n in enumerate(TILE_ELEMS):
            g = k * PER_CORE + off
            off += n
            ch = (g // SPATIAL) % C
            sum_v[ch] += vs[:, i].sum()
            # accum was sum of sign = 2*ones_tile - n_tile
            ones[ch] += (ct[:, i].sum() + n) / 2.0
    total = float(M_PER_CH)
    weight = np.where(ones > 0, total / np.maximum(ones, 1.0), EMPTY_WEIGHT)
    bce = -sum_v / total
    return np.asarray((weight * bce).mean(), dtype=np.float32)


def kernel(input, target):
    res = _run_device(input, target)
    return _epilogue(res.results)


# revision 11
# speedup vs baseline: 1.1951x; 1.1951x over previous
"""Trainium2 Bass kernel for CustomBCELoss.

Reference semantics (per torch BCELoss with per-channel weighting):
    p, t flattened channel-first to (C=3, M=8388608)
    ones[c]   = count_nonzero(t[c])
    weight[c] = M / max(ones[c], 1)  if ones[c] > 0 else 1000.0
    bce[c]    = -mean(t*max(log p, -100) + (1-t)*max(log1p(-p), -100))
    out       = mean(weight * bce)

Since t ∈ {0,1}, the per-element term is log|p + t - 1|, and with
p ∈ [1e-4, 1-1e-4] the -100 clamp never fires: |p + t - 1| >= ~6e-5.

Single-stream encoding: p > 0 always, so its fp32 sign bit is free. The
host packs t there (p'' = +p if t==1 else -p, a lossless re-encoding of
the (p, t) pair), halving the HBM stream to 12.6 MB/core vs streaming
(p, t) separately.

The per-element work is 3 logical passes (u-prep, count, log) over two
elementwise engines (DVE ~114 G elem/s, ACT ~131 G elem/s at fp32):
  DVE (all tiles): u = |p + t - 1| = (p'' < 0) + p''  -- ONE fused STT
      with src0 == src1 == p_t (the is_lt intermediate is the 1-t
      step); inner rounding 2^-24 -> ~3e-7 relative on the loss.
  ACT (all tiles): Ln(u) with fused per-partition accum_out.
  count (a full third pass) is SPLIT by tile between the engines:
      ACT tiles (early): Sign(p'') + accum_out; sign ∈ {-1,+1} is in
          the natural_log table set (no table switch); host recovers
          ones = (accum + n)/2 exactly. Sign/is_gt depend only on the
          DMA, so ACT front-loads its share while DVE builds a queue
          of u tiles; DVE takes the late tiles and pre-runs.
      DVE tiles (late): tensor_scalar is_gt + accum_out.
Stick to exactly this op set: plain (no-accum) tensor_scalar, fp32
tensor_tensor pair-products, PE matmuls, fp32r matmuls, and GpSimd DMA
each measured (directly or via the baseline's notes) a 1.2x static
clock derate on ALL engines. The accum variants at 1x full clock win.
Tiles open at 512 cols for a fast ramp, cruise at 2048, taper
1024/1024/1024/512/512 so the drain is short chains on small tiles.
A dummy Ln in the preamble pins the table set. Results ship in
readiness order. Tiles never cross an (n, c) half-block boundary, so
per-tile partials map 1:1 to channels on the host, which applies the
tiny weight/mean epilogue in float64.
"""

import numpy as np

import concourse.bacc as bacc
import concourse.bass as bass
import concourse.tile as tile
from concourse import mybir
from concourse.bass_utils import run_bass_kernel_spmd

N_CORES = 8
C = 3
SPATIAL = 128 * 128 * 128            # elements per (n, c) block
N_BATCH = 4
FULL = N_BATCH * C * SPATIAL         # 25_165_824 total elements
PER_CORE = FULL // N_CORES           # 3_145_728
P = 128
# Per-partition column counts per tile; sum must equal PER_CORE / P = 24576.
TILE_F = [512, 1536, 2048, 2048, 2048,
          2048, 2048, 2048, 2048,
          2048, 2048, 1024, 1024, 1024, 512, 512]
NTILES = len(TILE_F)
TILE_ELEMS = [P * f for f in TILE_F]
assert sum(TILE_ELEMS) == PER_CORE
HALF_BLOCK_COLS = (SPATIAL // 2) // P          # 8192 cols per half-block
M_PER_CH = FULL // C                 # 8_388_608
EMPTY_WEIGHT = 1000.0
VS_SPLIT = 13                        # bulk/tail split for the output DMAs
# Tiles whose count runs on DVE (is_gt+accum, late tiles); the rest
# count on ACT (Sign+accum, early tiles). 12288 columns each.
DVE_CNT_TILES = {4, 5, 9, 10, 11, 12, 13, 14, 15}

_NC_CACHE = None


def _build_nc():
    nc = bacc.Bacc(
        "TRN2", target_bir_lowering=False, debug=False, num_devices=N_CORES
    )
    p_in = nc.declare_dram_parameter(
        "p_in", [PER_CORE], mybir.dt.float32, isOutput=False
    )
    vsum_out = nc.declare_dram_parameter(
        "vsum", [P, NTILES], mybir.dt.float32, isOutput=True
    )
    cnt_out = nc.declare_dram_parameter(
        "cnt", [P, NTILES], mybir.dt.float32, isOutput=True
    )

    off = 0
    for f in TILE_F:
        assert off // HALF_BLOCK_COLS == (off + f - 1) // HALF_BLOCK_COLS
        off += f

    with tile.TileContext(nc) as tc:
        with (
            tc.tile_pool(name="pp", bufs=11) as p_pool,
            tc.tile_pool(name="up", bufs=8) as u_pool,
            tc.tile_pool(name="dp", bufs=2) as dump_pool,
            tc.tile_pool(name="res", bufs=1) as res_pool,
        ):
            vsum_t = res_pool.tile([P, NTILES], mybir.dt.float32)
            cnt_t = res_pool.tile([P, NTILES], mybir.dt.float32)
            # Dummy Ln pins the natural_log table set in the preamble
            # (it also contains Sign).
            warm_t = res_pool.tile([P, 1], mybir.dt.float32)
            nc.vector.memset(warm_t, 1.0)
            nc.scalar.activation(
                out=warm_t, in_=warm_t, func=mybir.ActivationFunctionType.Ln
            )
            off = 0
            for i, f in enumerate(TILE_F):
                n = P * f
                p_src = p_in[off : off + n].rearrange("(p f) -> p f", p=P)
                off += n
                p_t = p_pool.tile([P, f], mybir.dt.float32, tag="p")
                nc.sync.dma_start(out=p_t, in_=p_src)
                if i in DVE_CNT_TILES:
                    dump = dump_pool.tile([P, f], mybir.dt.bfloat16, tag="d")
                    nc.vector.tensor_scalar(
                        out=dump,
                        in0=p_t,
                        scalar1=0.0,
                        scalar2=None,
                        op0=mybir.AluOpType.is_gt,
                        op1=mybir.AluOpType.add,
                        accum_out=cnt_t[:, i : i + 1],
                    )
                else:
                    dump = dump_pool.tile([P, f], mybir.dt.bfloat16, tag="d")
                    nc.scalar.activation(
                        out=dump,
                        in_=p_t,
                        func=mybir.ActivationFunctionType.Sign,
                        accum_out=cnt_t[:, i : i + 1],
                    )
                # u = |p + t - 1| = (p'' < 0) + p'', fused STT (src0==src1).
                u_t = u_pool.tile([P, f], mybir.dt.float32, tag="u")
                nc.vector.scalar_tensor_tensor(
                    out=u_t,
                    in0=p_t,
                    scalar=0.0,
                    in1=p_t,
                    op0=mybir.AluOpType.is_lt,
                    op1=mybir.AluOpType.add,
                )
                nc.scalar.activation(
                    out=u_t,
                    in_=u_t,
                    func=mybir.ActivationFunctionType.Ln,
                    accum_out=vsum_t[:, i : i + 1],
                )
            # Ship results in readiness order so only a tiny vsum chunk
            # trails the last Ln.
            nc.sync.dma_start(
                out=cnt_out[:, :VS_SPLIT], in_=cnt_t[:, :VS_SPLIT]
            )
            nc.sync.dma_start(
                out=vsum_out[:, :VS_SPLIT], in_=vsum_t[:, :VS_SPLIT]
            )
            nc.sync.dma_start(
                out=cnt_out[:, VS_SPLIT:], in_=cnt_t[:, VS_SPLIT:]
            )
            nc.sync.dma_start(
                out=vsum_out[:, VS_SPLIT:], in_=vsum_t[:, VS_SPLIT:]
            )
    nc.compile()
    return nc


def _get_nc():
    global _NC_CACHE
    if _NC_CACHE is None:
        _NC_CACHE = _build_nc()
    return _NC_CACHE


def _pack(input, target):
    """Lossless (p, t) -> p'' re-encoding: t into p's free sign bit."""
    p_flat = np.ascontiguousarray(input, dtype=np.float32).reshape(-1)
    t_flat = np.ascontiguousarray(target, dtype=np.float32).reshape(-1)
    p_bits = p_flat.view(np.uint32)
    sign = np.where(t_flat == 0.0, np.uint32(0x80000000), np.uint32(0))
    return (p_bits | sign).view(np.float32)


def _run_device(input, target, **spmd_kwargs):
    packed = _pack(input, target)
    in_maps = []
    for k in range(N_CORES):
        sl = slice(k * PER_CORE, (k + 1) * PER_CORE)
        in_maps.append({"p_in": packed[sl]})
    return run_bass_kernel_spmd(nc=_get_nc(), in_maps=in_maps,
                                core_ids=list(range(N_CORES)), **spmd_kwargs)


def _epilogue(results):
    sum_v = np.zeros(C, dtype=np.float64)
    ones = np.zeros(C, dtype=np.float64)
    for k in range(N_CORES):
        vs = results[k]["vsum"].astype(np.float64)   # [P, NTILES]
        ct = results[k]["cnt"].astype(np.float64)    # [P, NTILES]
        off = 0
        for i, n in enumerate(TILE_ELEMS):
            g = k * PER_CORE + off
            off += n
            ch = (g // SPATIAL) % C
            sum_v[ch] += vs[:, i].sum()
            if i in DVE_CNT_TILES:
                ones[ch] += ct[:, i].sum()
            else:
                # accum was sum of sign = 2*ones_tile - n_tile
                ones[ch] += (ct[:, i].sum() + n) / 2.0
    total = float(M_PER_CH)
    weight = np.where(ones > 0, total / np.maximum(ones, 1.0), EMPTY_WEIGHT)
    bce = -sum_v / total
    return np.asarray((weight * bce).mean(), dtype=np.float32)


def kernel(input, target):
    res = _run_device(input, target)
    return _epilogue(res.results)
